# revision 12
# baseline (speedup 1.0000x reference)
"""Trainium2 Bass kernel for nn_BinnedLoss (tent-weighted 128-bin chi2 loss).

Self-contained 8-core SPMD program. Shards the N=16.7M sample axis across
cores. Per core, each sample contributes two (key, value) pairs
(kc+1 -> cp, kc -> cm); per-bin sums are computed by mask-reduce
(scalar_tensor_tensor is_equal*mult with accum_out) -- one instruction per
bin -- then collapsed across partitions with a single ones-matmul,
all-reduced, normalized, and turned into the chi2 scalar on every core.

kernel(**inputs) -> np.float32 scalar (shape ()).
"""
import os
import sys

sys.path.insert(0, "/opt/trn_rl_repo")
import numpy as np

N = 16777216
NCORES = 8
BINS = 128
P = 128
NSH = N // NCORES            # samples per core
FTOT_FULL = NSH // P         # 16384 free columns per core per array
MAGIC = 8388608.0            # 2^23 round-to-nearest trick


def _patches(mybir, tile):
    from concourse.vector_clock import ScopedClock

    def _patched(self, tick_clock, wait_clock):
        drain_inst = self.nc.sync.drain()
        wait_clock.add_sem_waits(
            drain_inst.ins, ScopedClock({None: tick_clock.global_clock})
        )
        si = drain_inst.ins.sync_info
        if si is not None and si.on_wait and len(si.on_wait) > 1:
            waits = list(si.on_wait)
            drain_inst.ins.sync_info = mybir.SyncInfo(
                on_wait=[waits[0]], on_update=list(si.on_update)
            )
            for w in waits[1:]:
                nop = self.nc.sync.nop()
                nop.ins.sync_info = mybir.SyncInfo(on_wait=[w], on_update=[])
        self.nc.all_engine_barrier()
        assert self.sems is not None
        popped = self.nc._tile_sem_poison_stack.pop()
        assert popped is self._sem_poison
        self.nc.clear_and_free_semaphores(list(self.sems.allocated().values()))
        self.nc.all_engine_barrier()

    tile.TileContext._drain_and_barrier = _patched


def _split_sync_waits(nc, mybir):
    """This walrus build allows <=1 sem-wait per instruction; hoist extras
    onto same-engine NOPs inserted just before the instruction."""
    counter = [0]
    for f in nc.m.functions:
        for bb in f.blocks:
            out = []
            dirty = False
            for inst in bb.instructions:
                si = inst.sync_info
                if si is not None and si.on_wait and len(si.on_wait) > 1:
                    waits = list(si.on_wait)
                    for w in waits[:-1]:
                        counter[0] += 1
                        nop = mybir.InstNoOp(
                            name=f"WSPLIT-{counter[0]}", ins=[], outs=[]
                        )
                        nop.engine = inst.engine
                        nop.sync_info = mybir.SyncInfo(on_wait=[w], on_update=[])
                        nc.register_instruction(nop, overwrite=True)
                        out.append(nop)
                    inst.sync_info = mybir.SyncInfo(
                        on_wait=[waits[-1]], on_update=list(si.on_update)
                    )
                    dirty = True
                out.append(inst)
            if dirty:
                bb.instructions = out


def build(ftot=FTOT_FULL, ncores=NCORES, repeat=1, half=None, fc=None):
    import concourse.bass as bass
    import concourse.mybir as mybir
    from concourse import tile

    _patches(mybir, tile)
    DT = mybir.dt
    AL = mybir.AluOpType
    ACT = mybir.ActivationFunctionType
    F32 = DT.float32
    BF16 = DT.bfloat16
    core_ids = list(range(ncores))
    HALF = half if half is not None else min(8192, ftot)   # cols per key pass
    FC = fc if fc is not None else min(2048, HALF)         # cols per prep chunk
    assert ftot % HALF == 0 and HALF % FC == 0
    NHALF = ftot // HALF

    nc = bass.Bass()
    sim_ext = nc.declare_dram_parameter("sim", [P, ftot], F32, isOutput=False)
    exp_ext = nc.declare_dram_parameter("exp", [P, ftot], F32, isOutput=False)
    w_ext = nc.declare_dram_parameter("w", [P, ftot], F32, isOutput=False)
    out_ext = nc.declare_dram_parameter("out", [1, 1], F32, isOutput=True)

    with tile.TileContext(nc) as tc:
        with (
            tc.tile_pool(name="const", bufs=1) as cpool,
            tc.tile_pool(name="dram", bufs=1, space="DRAM") as dram,
            tc.tile_pool(name="psum", bufs=1, space="PSUM") as psum,
        ):
            cc_a_in = dram.tile([1, 2], F32, name="cc_a_in")
            cc_a_out = dram.tile([1, 2], F32, name="cc_a_out")
            cc_h_in = dram.tile([1, 256], F32, name="cc_h_in")
            cc_h_out = dram.tile([1, 256], F32, name="cc_h_out")

            ones1 = cpool.tile([1, P], F32, name="ones1")
            nc.vector.memset(ones1[:], 1.0)
            ones128 = cpool.tile([P, 1], F32, name="ones128")
            nc.vector.memset(ones128[:], 1.0)

            # scalars: sc = [mn, step, inv, bias0, mn+step, delta]
            sc = cpool.tile([1, 6], F32, name="sc")
            bc = cpool.tile([P, 6], F32, name="bc")
            bcps = psum.tile([P, 6], F32, name="bcps", tag="bcps")
            ps1 = psum.tile([1, 256], F32, name="ps1", tag="ps1")

            # phase-B working set (reused across halves/arrays/reps)
            K2 = cpool.tile([P, 2 * HALF], BF16, name="K2")
            V2 = cpool.tile([P, 2 * HALF], BF16, name="V2")
            junk = cpool.tile([P, 2 * HALF], BF16, name="junk")
            CC = cpool.tile([P, 256], F32, name="CC")
            accH = [cpool.tile([P, BINS], F32, name=f"accH{h}") for h in range(2)]
            ghr = cpool.tile([1, 256], F32, name="ghr")
            gh = cpool.tile([1, 256], F32, name="gh")

            for rep in range(repeat):
                # ---------------- Phase A: global min/max ----------------
                with tc.tile_pool(name=f"pa{rep}", bufs=2) as pa:
                    CW = min(8192, ftot)
                    rmin = pa.tile([P, 1], F32, name="rmin", bufs=1)
                    rmax = pa.tile([P, 1], F32, name="rmax", bufs=1)
                    first = True
                    for arr in (sim_ext, exp_ext):
                        for c0 in range(0, ftot, CW):
                            ch = pa.tile([P, CW], F32, name="ch", tag="ch")
                            nc.sync.dma_start(ch[:], arr[:, c0:c0 + CW])
                            tmin = pa.tile([P, 1], F32, name="tmin", tag="tmin")
                            tmax = pa.tile([P, 1], F32, name="tmax", tag="tmax")
                            nc.vector.tensor_reduce(
                                tmin[:], ch[:], mybir.AxisListType.X, AL.min)
                            nc.vector.tensor_reduce(
                                tmax[:], ch[:], mybir.AxisListType.X, AL.max)
                            if first:
                                nc.vector.tensor_copy(rmin[:], tmin[:])
                                nc.vector.tensor_copy(rmax[:], tmax[:])
                                first = False
                            else:
                                nc.vector.tensor_tensor(rmin[:], rmin[:], tmin[:], AL.min)
                                nc.vector.tensor_tensor(rmax[:], rmax[:], tmax[:], AL.max)
                    pm = pa.tile([1, 2 * P], F32, name="pm", bufs=1)
                    nc.gpsimd.dma_start(pm[0:1, 0:P], rmax[:, 0:1])
                    nc.gpsimd.dma_start(pm[0:1, P:2 * P], rmin[:, 0:1])
                    pk = pa.tile([1, 2], F32, name="pk", bufs=1)
                    nc.vector.tensor_reduce(
                        pk[0:1, 0:1], pm[0:1, 0:P], mybir.AxisListType.X, AL.max)
                    nc.vector.tensor_reduce(
                        pk[0:1, 1:2], pm[0:1, P:2 * P], mybir.AxisListType.X, AL.min)
                    nc.vector.tensor_scalar_mul(pk[0:1, 1:2], pk[0:1, 1:2], -1.0)
                    nc.gpsimd.dma_start(cc_a_in[:], pk[:])
                    nc.gpsimd.collective_compute(
                        "AllReduce", AL.max, replica_groups=[core_ids],
                        ins=[cc_a_in.opt()], outs=[cc_a_out.opt()],
                    )
                    ga = pa.tile([1, 2], F32, name="ga", bufs=1)
                    nc.gpsimd.dma_start(ga[:], cc_a_out[:])
                    # ga = [mx, -mn]
                    nc.vector.tensor_scalar_mul(sc[0:1, 0:1], ga[0:1, 1:2], -1.0)
                    d_t = pa.tile([1, 1], F32, name="d_t", bufs=1)
                    nc.vector.tensor_tensor(d_t[:], ga[0:1, 0:1], sc[0:1, 0:1], AL.subtract)
                    nc.vector.tensor_scalar_mul(
                        sc[0:1, 1:2], d_t[:], float(np.float32(1.0) / np.float32(127.0)))
                    nc.vector.reciprocal(sc[0:1, 2:3], sc[0:1, 1:2])
                    nc.vector.scalar_tensor_tensor(
                        sc[0:1, 3:4], sc[0:1, 0:1], -1.0, sc[0:1, 2:3],
                        AL.mult, AL.mult)
                    nc.vector.tensor_tensor(
                        sc[0:1, 4:5], sc[0:1, 0:1], sc[0:1, 1:2], AL.add)
                    nc.vector.tensor_scalar_mul(sc[0:1, 5:6], d_t[:], 0.0078125)
                    nc.tensor.matmul(bcps[:], ones1[:], sc[0:1, :],
                                     start=True, stop=True)
                    nc.vector.tensor_copy(bc[:], bcps[:])

                # ---------------- Phase B: mask-reduce histograms ----------------
                with tc.tile_pool(name=f"pb{rep}", bufs=1) as pb:
                    t = lambda nm: pb.tile([P, FC], F32, name=nm, tag=nm)
                    for ai, (arr, weighted) in enumerate(
                            ((sim_ext, True), (exp_ext, False))):
                        for h in range(NHALF):
                            for ci in range(HALF // FC):
                                c0 = h * HALF + ci * FC
                                k0 = ci * FC          # col offset in K2/V2
                                x = t("x")
                                nc.sync.dma_start(x[:], arr[:, c0:c0 + FC])
                                if weighted:
                                    wt = t("wt")
                                    nc.sync.dma_start(wt[:], w_ext[:, c0:c0 + FC])
                                u = t("u")
                                nc.scalar.activation(
                                    u[:], x[:], ACT.Identity,
                                    bias=bc[:, 3:4], scale=bc[:, 2:3])
                                kc = t("kc")
                                nc.vector.tensor_scalar(
                                    kc[:], u[:], MAGIC, -MAGIC, AL.add, AL.add)
                                s1 = t("s1")
                                nc.vector.tensor_tensor(s1[:], kc[:], u[:], AL.is_gt)
                                nc.vector.tensor_tensor(kc[:], kc[:], s1[:], AL.subtract)
                                nc.vector.tensor_scalar(
                                    kc[:], kc[:], 0.0, 126.0, AL.max, AL.min)
                                hk = t("hk")
                                nc.scalar.activation(
                                    hk[:], kc[:], ACT.Identity,
                                    bias=bc[:, 0:1], scale=bc[:, 1:2])
                                # s1 = in_iv = (x >= hk) & (hk + step > x)
                                s2 = t("s2")
                                nc.vector.tensor_tensor(s1[:], x[:], hk[:], AL.is_ge)
                                nc.vector.scalar_tensor_tensor(
                                    s2[:], hk[:], bc[:, 1:2], x[:], AL.add, AL.is_gt)
                                nc.vector.tensor_tensor(s1[:], s1[:], s2[:], AL.mult)
                                # masks with kc-range conditions
                                mp = t("mp")
                                nc.vector.scalar_tensor_tensor(
                                    mp[:], kc[:], 125.5, s1[:], AL.is_lt, AL.mult)
                                mm = t("mm")
                                nc.vector.scalar_tensor_tensor(
                                    mm[:], kc[:], 0.5, s1[:], AL.is_gt, AL.mult)
                                # cp = (x - hk) [*w] * mp -> V2[:, k0:]
                                cp = t("cp")
                                nc.vector.tensor_tensor(cp[:], x[:], hk[:], AL.subtract)
                                if weighted:
                                    nc.vector.tensor_tensor(cp[:], cp[:], wt[:], AL.mult)
                                nc.vector.tensor_tensor(
                                    V2[:, k0:k0 + FC], cp[:], mp[:], AL.mult)
                                # cm = (hk + step - x) [*w] * mm -> V2[:, HALF+k0:]
                                cm = t("cm")
                                nc.vector.scalar_tensor_tensor(
                                    cm[:], hk[:], bc[:, 1:2], x[:], AL.add, AL.subtract)
                                if weighted:
                                    nc.vector.tensor_tensor(cm[:], cm[:], wt[:], AL.mult)
                                nc.vector.tensor_tensor(
                                    V2[:, HALF + k0:HALF + k0 + FC], cm[:], mm[:], AL.mult)
                                # keys
                                nc.vector.tensor_scalar_add(
                                    K2[:, k0:k0 + FC], kc[:], 1.0)
                                nc.vector.tensor_copy(
                                    K2[:, HALF + k0:HALF + k0 + FC], kc[:])
                            # mask-reduce over this half
                            acc = accH[h % 2]
                            nc.vector.memset(acc[:], 0.0)
                            for b in range(1, 127):
                                nc.vector.scalar_tensor_tensor(
                                    junk[:], K2[:], float(b), V2[:],
                                    AL.is_equal, AL.mult,
                                    accum_out=acc[:, b:b + 1])
                        if NHALF == 1:
                            nc.vector.tensor_copy(
                                CC[:, ai * BINS:(ai + 1) * BINS], accH[0][:])
                        else:
                            nc.vector.tensor_tensor(
                                CC[:, ai * BINS:(ai + 1) * BINS],
                                accH[0][:], accH[1][:], AL.add)
                    # partition collapse: [1,256] = ones.T @ CC
                    nc.tensor.matmul(ps1[:], ones128[:], CC[:],
                                     start=True, stop=True)
                    nc.vector.tensor_copy(ghr[:], ps1[:])

                # ---------------- Phase C: all-reduce + chi2 ----------------
                with tc.tile_pool(name=f"pc{rep}", bufs=1) as pc:
                    nc.gpsimd.dma_start(cc_h_in[:], ghr[:])
                    nc.gpsimd.collective_compute(
                        "AllReduce", AL.add, replica_groups=[core_ids],
                        ins=[cc_h_in.opt()], outs=[cc_h_out.opt()],
                    )
                    nc.gpsimd.dma_start(gh[:], cc_h_out[:])
                    for ai in range(2):
                        hist = gh[0:1, ai * BINS:(ai + 1) * BINS]
                        ssum = pc.tile([1, 1], F32, name=f"ssum{ai}")
                        nc.vector.tensor_reduce(
                            ssum[:], hist, mybir.AxisListType.X, AL.add)
                        nc.vector.tensor_tensor(ssum[:], ssum[:], sc[0:1, 5:6], AL.mult)
                        nc.vector.reciprocal(ssum[:], ssum[:])
                        nc.vector.tensor_scalar(
                            hist, hist, ssum[0:1, 0:1], None, AL.mult)
                    dif = pc.tile([1, BINS], F32, name="dif")
                    nc.vector.tensor_tensor(
                        dif[:], gh[0:1, 0:BINS], gh[0:1, BINS:2 * BINS], AL.subtract)
                    nc.vector.tensor_tensor(dif[:], dif[:], dif[:], AL.mult)
                    chi = pc.tile([1, 1], F32, name="chi")
                    nc.vector.tensor_reduce(
                        chi[:], dif[:], mybir.AxisListType.X, AL.add)
                    nc.gpsimd.dma_start(out_ext[:], chi[:])

    _split_sync_waits(nc, __import__("concourse.mybir", fromlist=["x"]))
    return nc


_CACHE = {}


def _get_nc(repeat):
    key = repeat
    if key not in _CACHE:
        _CACHE[key] = build(repeat=repeat)
    return _CACHE[key]


def kernel(**inputs):
    sim = np.ascontiguousarray(inputs["sim_observable"], dtype=np.float32)
    exp = np.ascontiguousarray(inputs["exp_observable"], dtype=np.float32)
    w = np.ascontiguousarray(inputs["weights"], dtype=np.float32)
    assert sim.shape == (N,) and exp.shape == (N,) and w.shape == (N,)

    from concourse.bass_utils import run_bass_kernel_spmd

    repeat = int(os.environ.get("BASS_HIST_REPEAT", "1"))
    nc = _get_nc(repeat)
    sim_s = sim.reshape(NCORES, P, FTOT_FULL)
    exp_s = exp.reshape(NCORES, P, FTOT_FULL)
    w_s = w.reshape(NCORES, P, FTOT_FULL)
    in_maps = [
        {"sim": sim_s[c], "exp": exp_s[c], "w": w_s[c]} for c in range(NCORES)
    ]
    res = run_bass_kernel_spmd(nc, in_maps, list(range(NCORES)))
    val = res.results[0]["out"][0, 0]
    return np.asarray(val, dtype=np.float32).reshape(())


# revision 25
# speedup vs baseline: 2.5401x; 2.5401x over previous
"""Trainium2 Bass kernel for nn_BinnedLoss (tent-weighted 128-bin chi2 loss).

Self-contained 8-core SPMD program. Shards the N=16.7M sample axis across
cores. Per core, each sample contributes two (key, value) pairs
(kc+1 -> cp, kc -> cm); per-bin sums are computed by mask-reduce
(scalar_tensor_tensor is_equal*mult with accum_out) -- one instruction per
bin -- then collapsed across partitions with a single ones-matmul,
all-reduced, normalized, and turned into the chi2 scalar on every core.

kernel(**inputs) -> np.float32 scalar (shape ()).
"""
import os
import sys

sys.path.insert(0, "/opt/trn_rl_repo")
import numpy as np

N = 16777216
NCORES = 8
BINS = 128
P = 128
NSH = N // NCORES            # samples per core
FTOT_FULL = NSH // P         # 16384 free columns per core per array
MAGIC = 8388608.0            # 2^23 round-to-nearest trick


def _patches(mybir, tile):
    from concourse.vector_clock import ScopedClock

    def _patched(self, tick_clock, wait_clock):
        drain_inst = self.nc.sync.drain()
        wait_clock.add_sem_waits(
            drain_inst.ins, ScopedClock({None: tick_clock.global_clock})
        )
        si = drain_inst.ins.sync_info
        if si is not None and si.on_wait and len(si.on_wait) > 1:
            waits = list(si.on_wait)
            drain_inst.ins.sync_info = mybir.SyncInfo(
                on_wait=[waits[0]], on_update=list(si.on_update)
            )
            for w in waits[1:]:
                nop = self.nc.sync.nop()
                nop.ins.sync_info = mybir.SyncInfo(on_wait=[w], on_update=[])
        self.nc.all_engine_barrier()
        assert self.sems is not None
        popped = self.nc._tile_sem_poison_stack.pop()
        assert popped is self._sem_poison
        self.nc.clear_and_free_semaphores(list(self.sems.allocated().values()))
        self.nc.all_engine_barrier()

    tile.TileContext._drain_and_barrier = _patched


def _split_sync_waits(nc, mybir, strip_same_engine=True):
    """Two fixups for this walrus/runtime:
    1. Drop same-engine waits (the engine is in-order and every DVE op is
       followed by an implicit pipeline DRAIN, so engine-vs-own-sem waits are
       redundant) -- wait-carrying instructions are ~10x slower here.
    2. The walrus build allows <=1 sem-wait per instruction; hoist extras
       onto same-engine NOPs inserted just before the instruction."""
    eng_sem = {}
    counter = [0]
    for f in nc.m.functions:
        for bb in f.blocks:
            out = []
            dirty = False
            for inst in bb.instructions:
                si = inst.sync_info
                pref = eng_sem.get(inst.engine) if strip_same_engine else None
                if si is not None and si.on_wait and pref is not None:
                    kept = [
                        w for w in si.on_wait
                        if not (w.ant_name or "").startswith(pref + "_")
                    ]
                    if len(kept) != len(si.on_wait):
                        inst.sync_info = mybir.SyncInfo(
                            on_wait=kept, on_update=list(si.on_update))
                        si = inst.sync_info
                        dirty = True
                if si is not None and si.on_wait and len(si.on_wait) > 1:
                    waits = list(si.on_wait)
                    for w in waits[:-1]:
                        counter[0] += 1
                        nop = mybir.InstNoOp(
                            name=f"WSPLIT-{counter[0]}", ins=[], outs=[]
                        )
                        nop.engine = inst.engine
                        nop.sync_info = mybir.SyncInfo(on_wait=[w], on_update=[])
                        nc.register_instruction(nop, overwrite=True)
                        out.append(nop)
                    inst.sync_info = mybir.SyncInfo(
                        on_wait=[waits[-1]], on_update=list(si.on_update)
                    )
                    dirty = True
                out.append(inst)
            if dirty:
                bb.instructions = out


def build(ftot=FTOT_FULL, ncores=NCORES, repeat=1, half=None, fc=None, strip_waits=True):
    import concourse.bass as bass
    import concourse.mybir as mybir
    from concourse import tile

    _patches(mybir, tile)
    DT = mybir.dt
    AL = mybir.AluOpType
    ACT = mybir.ActivationFunctionType
    F32 = DT.float32
    BF16 = DT.bfloat16
    core_ids = list(range(ncores))
    HALF = half if half is not None else ftot              # cols per key pass
    FC = fc if fc is not None else min(512, HALF)          # cols per prep chunk
    assert ftot % HALF == 0 and HALF % FC == 0
    NHALF = ftot // HALF

    nc = bass.Bass()
    sim_ext = nc.declare_dram_parameter("sim", [P, ftot], F32, isOutput=False)
    exp_ext = nc.declare_dram_parameter("exp", [P, ftot], F32, isOutput=False)
    w_ext = nc.declare_dram_parameter("w", [P, ftot], F32, isOutput=False)
    out_ext = nc.declare_dram_parameter("out", [1, 1], F32, isOutput=True)

    with tile.TileContext(nc) as tc:
        with (
            tc.tile_pool(name="const", bufs=1) as cpool,
            tc.tile_pool(name="dram", bufs=1, space="DRAM") as dram,
            tc.tile_pool(name="psum", bufs=1, space="PSUM") as psum,
        ):
            cc_a_in = dram.tile([1, 2], F32, name="cc_a_in")
            cc_a_out = dram.tile([1, 2], F32, name="cc_a_out")
            cc_h_in = dram.tile([1, 256], F32, name="cc_h_in")
            cc_h_out = dram.tile([1, 256], F32, name="cc_h_out")

            ones1 = cpool.tile([1, P], F32, name="ones1")
            nc.vector.memset(ones1[:], 1.0)
            ones128 = cpool.tile([P, 1], F32, name="ones128")
            nc.vector.memset(ones128[:], 1.0)
            bcol_i = cpool.tile([P, BINS], DT.int32, name="bcol_i")
            nc.gpsimd.iota(bcol_i[:], [[1, BINS]], channel_multiplier=0)
            bcol = cpool.tile([P, BINS], F32, name="bcol")
            nc.vector.tensor_copy(bcol[:], bcol_i[:])

            # scalars: sc = [mn, step, inv, bias0, mn+step, delta]
            sc = cpool.tile([1, 6], F32, name="sc")
            bc = cpool.tile([P, 6], F32, name="bc")
            bcps = psum.tile([P, 6], F32, name="bcps", tag="bcps")
            ps1 = psum.tile([1, 256], F32, name="ps1", tag="ps1")

            # phase-B working set (reused across arrays/reps)
            K2 = cpool.tile([P, 2 * HALF], BF16, name="K2")
            V2 = cpool.tile([P, 2 * HALF], BF16, name="V2")
            CC = cpool.tile([P, 256], F32, name="CC")
            accH = [cpool.tile([P, BINS], F32, name=f"accH{h}") for h in range(2)]
            ghr = cpool.tile([1, 256], F32, name="ghr")
            gh = cpool.tile([1, 256], F32, name="gh")

            for rep in range(1):
                # ---------------- Phase A: global min/max ----------------
                with tc.tile_pool(name=f"pa{rep}", bufs=2) as pa:
                    CW = min(8192, ftot)
                    rmin = pa.tile([P, 1], F32, name="rmin", bufs=1)
                    rmax = pa.tile([P, 1], F32, name="rmax", bufs=1)
                    first = True
                    for krep in range(repeat):
                      for arr in (sim_ext, exp_ext):
                        for c0 in range(0, ftot, CW):
                            ch = pa.tile([P, CW], F32, name="ch", tag="ch")
                            nc.sync.dma_start(ch[:], arr[:, c0:c0 + CW])
                            tmin = pa.tile([P, 1], F32, name="tmin", tag="tmin")
                            tmax = pa.tile([P, 1], F32, name="tmax", tag="tmax")
                            nc.vector.tensor_reduce(
                                tmin[:], ch[:], mybir.AxisListType.X, AL.min)
                            nc.vector.tensor_reduce(
                                tmax[:], ch[:], mybir.AxisListType.X, AL.max)
                            if first:
                                nc.vector.tensor_copy(rmin[:], tmin[:])
                                nc.vector.tensor_copy(rmax[:], tmax[:])
                                first = False
                            else:
                                nc.vector.tensor_tensor(rmin[:], rmin[:], tmin[:], AL.min)
                                nc.vector.tensor_tensor(rmax[:], rmax[:], tmax[:], AL.max)
                    pm = pa.tile([1, 2 * P], F32, name="pm", bufs=1)
                    nc.gpsimd.dma_start(pm[0:1, 0:P], rmax[:, 0:1])
                    nc.gpsimd.dma_start(pm[0:1, P:2 * P], rmin[:, 0:1])
                    pk = pa.tile([1, 2], F32, name="pk", bufs=1)
                    nc.vector.tensor_reduce(
                        pk[0:1, 0:1], pm[0:1, 0:P], mybir.AxisListType.X, AL.max)
                    nc.vector.tensor_reduce(
                        pk[0:1, 1:2], pm[0:1, P:2 * P], mybir.AxisListType.X, AL.min)
                    nc.vector.tensor_scalar_mul(pk[0:1, 1:2], pk[0:1, 1:2], -1.0)
                    nc.gpsimd.dma_start(cc_a_in[:], pk[:])
                    nc.gpsimd.collective_compute(
                        "AllReduce", AL.max, replica_groups=[core_ids],
                        ins=[cc_a_in.opt()], outs=[cc_a_out.opt()],
                    )
                    ga = pa.tile([1, 2], F32, name="ga", bufs=1)
                    nc.gpsimd.dma_start(ga[:], cc_a_out[:])
                    # ga = [mx, -mn]
                    nc.vector.tensor_scalar_mul(sc[0:1, 0:1], ga[0:1, 1:2], -1.0)
                    d_t = pa.tile([1, 1], F32, name="d_t", bufs=1)
                    nc.vector.tensor_tensor(d_t[:], ga[0:1, 0:1], sc[0:1, 0:1], AL.subtract)
                    nc.vector.tensor_scalar_mul(
                        sc[0:1, 1:2], d_t[:], float(np.float32(1.0) / np.float32(127.0)))
                    nc.vector.reciprocal(sc[0:1, 2:3], sc[0:1, 1:2])
                    nc.vector.scalar_tensor_tensor(
                        sc[0:1, 3:4], sc[0:1, 0:1], -1.0, sc[0:1, 2:3],
                        AL.mult, AL.mult)
                    nc.vector.tensor_tensor(
                        sc[0:1, 4:5], sc[0:1, 0:1], sc[0:1, 1:2], AL.add)
                    nc.vector.tensor_scalar_mul(sc[0:1, 5:6], d_t[:], 0.0078125)
                    nc.tensor.matmul(bcps[:], ones1[:], sc[0:1, :],
                                     start=True, stop=True)
                    nc.vector.tensor_copy(bc[:], bcps[:])

                # ---------------- Phase B: mask-reduce histograms ----------------
                for ai, (arr, weighted) in enumerate(
                        ((sim_ext, True), (exp_ext, False))):
                    with tc.tile_pool(name=f"pp{rep}_{ai}", bufs=1) as pp:
                        t = lambda nm: pp.tile([P, FC], F32, name=nm, tag=nm)
                        x = t("x")
                        wt = t("wt")
                        u = t("u")
                        kc = t("kc")
                        s1 = t("s1")
                        s2 = t("s2")
                        hk = t("hk")
                        mp = t("mp")
                        mm = t("mm")
                        cp = t("cp")
                        cm = t("cm")
                        bst = lambda nm: pp.tile([P, FC], BF16, name=nm, tag=nm)
                        vps = bst("vps")
                        vms = bst("vms")
                        khs = bst("khs")
                        kls = bst("kls")
                        with tc.For_i(0, HALF, FC, name=f"prep{rep}_{ai}") as iv:
                            nc.sync.dma_start(x[:], arr[:, bass.ds(iv, FC)])
                            if weighted:
                                nc.sync.dma_start(wt[:], w_ext[:, bass.ds(iv, FC)])
                            # body replicated `repeat` times (idempotent) for
                            # timing via the R-slope method
                            for krep in range(repeat):
                                nc.scalar.activation(
                                    u[:], x[:], ACT.Identity,
                                    bias=bc[:, 3:4], scale=bc[:, 2:3])
                                nc.vector.tensor_scalar(
                                    kc[:], u[:], MAGIC, -MAGIC, AL.add, AL.add)
                                nc.vector.tensor_tensor(s1[:], kc[:], u[:], AL.is_gt)
                                nc.vector.tensor_tensor(kc[:], kc[:], s1[:], AL.subtract)
                                nc.vector.tensor_scalar(
                                    kc[:], kc[:], 0.0, 126.0, AL.max, AL.min)
                                nc.scalar.activation(
                                    hk[:], kc[:], ACT.Identity,
                                    bias=bc[:, 0:1], scale=bc[:, 1:2])
                                nc.vector.tensor_tensor(s1[:], x[:], hk[:], AL.is_ge)
                                nc.vector.scalar_tensor_tensor(
                                    s2[:], hk[:], bc[:, 1:2], x[:], AL.add, AL.is_gt)
                                nc.vector.tensor_tensor(s1[:], s1[:], s2[:], AL.mult)
                                nc.vector.scalar_tensor_tensor(
                                    mp[:], kc[:], 125.5, s1[:], AL.is_lt, AL.mult)
                                nc.vector.scalar_tensor_tensor(
                                    mm[:], kc[:], 0.5, s1[:], AL.is_gt, AL.mult)
                                nc.vector.tensor_tensor(cp[:], x[:], hk[:], AL.subtract)
                                if weighted:
                                    nc.vector.tensor_tensor(cp[:], cp[:], wt[:], AL.mult)
                                nc.vector.tensor_tensor(vps[:], cp[:], mp[:], AL.mult)
                                nc.vector.scalar_tensor_tensor(
                                    cm[:], hk[:], bc[:, 1:2], x[:], AL.add, AL.subtract)
                                if weighted:
                                    nc.vector.tensor_tensor(cm[:], cm[:], wt[:], AL.mult)
                                nc.vector.tensor_tensor(vms[:], cm[:], mm[:], AL.mult)
                                nc.vector.tensor_scalar_add(khs[:], kc[:], 1.0)
                                nc.vector.tensor_copy(kls[:], kc[:])
                            nc.vector.tensor_copy(V2[:, bass.ds(iv, FC)], vps[:])
                            nc.vector.tensor_copy(V2[:, bass.ds(iv + HALF, FC)], vms[:])
                            nc.vector.tensor_copy(K2[:, bass.ds(iv, FC)], khs[:])
                            nc.vector.tensor_copy(K2[:, bass.ds(iv + HALF, FC)], kls[:])
                    with tc.tile_pool(name=f"pj{rep}_{ai}", bufs=1) as pj:
                        junk = pj.tile([P, HALF], BF16, name="junk")
                        bcs = pj.tile([P, 1], F32, name="bcs")
                        as0 = pj.tile([P, 1], F32, name="as0")
                        as1 = pj.tile([P, 1], F32, name="as1")
                        nc.vector.memset(accH[0][:], 0.0)
                        nc.vector.memset(accH[1][:], 0.0)
                        with tc.For_i(0, BINS, name=f"bins{rep}_{ai}") as bv:
                            nc.vector.tensor_copy(bcs[:], bcol[:, bass.ds(bv, 1)])
                            for krep in range(repeat):
                                nc.vector.scalar_tensor_tensor(
                                    junk[:], K2[:, 0:HALF], bcs[:, 0:1],
                                    V2[:, 0:HALF], AL.is_equal, AL.mult,
                                    accum_out=as0[:, 0:1])
                                nc.vector.scalar_tensor_tensor(
                                    junk[:], K2[:, HALF:2 * HALF], bcs[:, 0:1],
                                    V2[:, HALF:2 * HALF], AL.is_equal, AL.mult,
                                    accum_out=as1[:, 0:1])
                            nc.vector.tensor_copy(accH[0][:, bass.ds(bv, 1)], as0[:])
                            nc.vector.tensor_copy(accH[1][:, bass.ds(bv, 1)], as1[:])
                    nc.vector.tensor_tensor(
                        CC[:, ai * BINS:(ai + 1) * BINS],
                        accH[0][:], accH[1][:], AL.add)
                # partition collapse: [1,256] = ones.T @ CC
                nc.tensor.matmul(ps1[:], ones128[:], CC[:],
                                 start=True, stop=True)
                nc.vector.tensor_copy(ghr[:], ps1[:])

                # ---------------- Phase C: all-reduce + chi2 ----------------
                with tc.tile_pool(name=f"pc{rep}", bufs=1) as pc:
                    nc.gpsimd.dma_start(cc_h_in[:], ghr[:])
                    nc.gpsimd.collective_compute(
                        "AllReduce", AL.add, replica_groups=[core_ids],
                        ins=[cc_h_in.opt()], outs=[cc_h_out.opt()],
                    )
                    nc.gpsimd.dma_start(gh[:], cc_h_out[:])
                    for ai in range(2):
                        hist = gh[0:1, ai * BINS:(ai + 1) * BINS]
                        ssum = pc.tile([1, 1], F32, name=f"ssum{ai}")
                        nc.vector.tensor_reduce(
                            ssum[:], hist, mybir.AxisListType.X, AL.add)
                        nc.vector.tensor_tensor(ssum[:], ssum[:], sc[0:1, 5:6], AL.mult)
                        nc.vector.reciprocal(ssum[:], ssum[:])
                        nc.vector.tensor_scalar(
                            hist, hist, ssum[0:1, 0:1], None, AL.mult)
                    dif = pc.tile([1, BINS], F32, name="dif")
                    nc.vector.tensor_tensor(
                        dif[:], gh[0:1, 0:BINS], gh[0:1, BINS:2 * BINS], AL.subtract)
                    nc.vector.tensor_tensor(dif[:], dif[:], dif[:], AL.mult)
                    chi = pc.tile([1, 1], F32, name="chi")
                    nc.vector.tensor_reduce(
                        chi[:], dif[:], mybir.AxisListType.X, AL.add)
                    nc.gpsimd.dma_start(out_ext[:], chi[:])

    _split_sync_waits(nc, __import__("concourse.mybir", fromlist=["x"]),
                      strip_same_engine=strip_waits)
    return nc


_CACHE = {}


def _get_nc(repeat):
    key = repeat
    if key not in _CACHE:
        _CACHE[key] = build(repeat=repeat)
    return _CACHE[key]


def kernel(**inputs):
    sim = np.ascontiguousarray(inputs["sim_observable"], dtype=np.float32)
    exp = np.ascontiguousarray(inputs["exp_observable"], dtype=np.float32)
    w = np.ascontiguousarray(inputs["weights"], dtype=np.float32)
    assert sim.shape == (N,) and exp.shape == (N,) and w.shape == (N,)

    from concourse.bass_utils import run_bass_kernel_spmd

    repeat = int(os.environ.get("BASS_HIST_REPEAT", "1"))
    nc = _get_nc(repeat)
    sim_s = sim.reshape(NCORES, P, FTOT_FULL)
    exp_s = exp.reshape(NCORES, P, FTOT_FULL)
    w_s = w.reshape(NCORES, P, FTOT_FULL)
    in_maps = [
        {"sim": sim_s[c], "exp": exp_s[c], "w": w_s[c]} for c in range(NCORES)
    ]
    res = run_bass_kernel_spmd(nc, in_maps, list(range(NCORES)))
    val = res.results[0]["out"][0, 0]
    return np.asarray(val, dtype=np.float32).reshape(())


# revision 27
# speedup vs baseline: 5.6638x; 2.2297x over previous
"""Trainium2 Bass kernel for nn_BinnedLoss (tent-weighted 128-bin chi2 loss).

Self-contained 8-core SPMD program. Shards the N=16.7M sample axis across
cores. Per core, each sample contributes two (key, value) pairs
(kc+1 -> cp, kc -> cm); per-bin sums are computed by mask-reduce
(scalar_tensor_tensor is_equal*mult with accum_out) -- one instruction per
bin -- then collapsed across partitions with a single ones-matmul,
all-reduced, normalized, and turned into the chi2 scalar on every core.

kernel(**inputs) -> np.float32 scalar (shape ()).
"""
import os
import sys

sys.path.insert(0, "/opt/trn_rl_repo")
import numpy as np

N = 16777216
NCORES = 8
BINS = 128
P = 128
NSH = N // NCORES            # samples per core
FTOT_FULL = NSH // P         # 16384 free columns per core per array
MAGIC = 8388608.0            # 2^23 round-to-nearest trick


def _patches(mybir, tile):
    from concourse.vector_clock import ScopedClock

    def _patched(self, tick_clock, wait_clock):
        drain_inst = self.nc.sync.drain()
        wait_clock.add_sem_waits(
            drain_inst.ins, ScopedClock({None: tick_clock.global_clock})
        )
        si = drain_inst.ins.sync_info
        if si is not None and si.on_wait and len(si.on_wait) > 1:
            waits = list(si.on_wait)
            drain_inst.ins.sync_info = mybir.SyncInfo(
                on_wait=[waits[0]], on_update=list(si.on_update)
            )
            for w in waits[1:]:
                nop = self.nc.sync.nop()
                nop.ins.sync_info = mybir.SyncInfo(on_wait=[w], on_update=[])
        self.nc.all_engine_barrier()
        assert self.sems is not None
        popped = self.nc._tile_sem_poison_stack.pop()
        assert popped is self._sem_poison
        self.nc.clear_and_free_semaphores(list(self.sems.allocated().values()))
        self.nc.all_engine_barrier()

    tile.TileContext._drain_and_barrier = _patched


def _split_sync_waits(nc, mybir, strip_same_engine=True):
    """Two fixups for this walrus/runtime:
    1. Drop same-engine waits (the engine is in-order and every DVE op is
       followed by an implicit pipeline DRAIN, so engine-vs-own-sem waits are
       redundant) -- wait-carrying instructions are ~10x slower here.
    2. The walrus build allows <=1 sem-wait per instruction; hoist extras
       onto same-engine NOPs inserted just before the instruction."""
    eng_sem = {}
    counter = [0]
    for f in nc.m.functions:
        for bb in f.blocks:
            out = []
            dirty = False
            for inst in bb.instructions:
                si = inst.sync_info
                pref = eng_sem.get(inst.engine) if strip_same_engine else None
                if si is not None and si.on_wait and pref is not None:
                    kept = [
                        w for w in si.on_wait
                        if not (w.ant_name or "").startswith(pref + "_")
                    ]
                    if len(kept) != len(si.on_wait):
                        inst.sync_info = mybir.SyncInfo(
                            on_wait=kept, on_update=list(si.on_update))
                        si = inst.sync_info
                        dirty = True
                if si is not None and si.on_wait and len(si.on_wait) > 1:
                    waits = list(si.on_wait)
                    for w in waits[:-1]:
                        counter[0] += 1
                        nop = mybir.InstNoOp(
                            name=f"WSPLIT-{counter[0]}", ins=[], outs=[]
                        )
                        nop.engine = inst.engine
                        nop.sync_info = mybir.SyncInfo(on_wait=[w], on_update=[])
                        nc.register_instruction(nop, overwrite=True)
                        out.append(nop)
                    inst.sync_info = mybir.SyncInfo(
                        on_wait=[waits[-1]], on_update=list(si.on_update)
                    )
                    dirty = True
                out.append(inst)
            if dirty:
                bb.instructions = out


def build(ftot=FTOT_FULL, ncores=NCORES, repeat=1, half=None, fc=None, strip_waits=True,
          repeat_prep=None, repeat_bins=None, repeat_pa=None):
    import concourse.bass as bass
    import concourse.mybir as mybir
    from concourse import tile

    _patches(mybir, tile)
    DT = mybir.dt
    AL = mybir.AluOpType
    ACT = mybir.ActivationFunctionType
    F32 = DT.float32
    BF16 = DT.bfloat16
    core_ids = list(range(ncores))
    R_PREP = repeat_prep if repeat_prep is not None else repeat
    R_BINS = repeat_bins if repeat_bins is not None else repeat
    R_PA = repeat_pa if repeat_pa is not None else repeat
    HALF = half if half is not None else ftot              # cols per key pass
    FC = fc if fc is not None else min(1024, HALF)         # cols per prep chunk
    assert ftot % HALF == 0 and HALF % FC == 0
    NHALF = ftot // HALF

    nc = bass.Bass()
    sim_ext = nc.declare_dram_parameter("sim", [P, ftot], F32, isOutput=False)
    exp_ext = nc.declare_dram_parameter("exp", [P, ftot], F32, isOutput=False)
    w_ext = nc.declare_dram_parameter("w", [P, ftot], F32, isOutput=False)
    out_ext = nc.declare_dram_parameter("out", [1, 1], F32, isOutput=True)

    with tile.TileContext(nc) as tc:
        with (
            tc.tile_pool(name="const", bufs=1) as cpool,
            tc.tile_pool(name="dram", bufs=1, space="DRAM") as dram,
            tc.tile_pool(name="psum", bufs=1, space="PSUM") as psum,
        ):
            cc_a_in = dram.tile([1, 2], F32, name="cc_a_in")
            cc_a_out = dram.tile([1, 2], F32, name="cc_a_out")
            cc_h_in = dram.tile([1, 256], F32, name="cc_h_in")
            cc_h_out = dram.tile([1, 256], F32, name="cc_h_out")

            ones1 = cpool.tile([1, P], F32, name="ones1")
            nc.vector.memset(ones1[:], 1.0)
            ones128 = cpool.tile([P, 1], F32, name="ones128")
            nc.vector.memset(ones128[:], 1.0)
            bcol_i = cpool.tile([P, BINS], DT.int32, name="bcol_i")
            nc.gpsimd.iota(bcol_i[:], [[1, BINS]], channel_multiplier=0)
            bcol = cpool.tile([P, BINS], F32, name="bcol")
            nc.vector.tensor_copy(bcol[:], bcol_i[:])

            # scalars: sc = [mn, step, inv, bias0, mn+step, delta]
            sc = cpool.tile([1, 6], F32, name="sc")
            bc = cpool.tile([P, 6], F32, name="bc")
            bcps = psum.tile([P, 6], F32, name="bcps", tag="bcps")
            ps1 = psum.tile([1, 256], F32, name="ps1", tag="ps1")

            # phase-B working set (reused across arrays/reps)
            K2 = cpool.tile([P, 2 * HALF], BF16, name="K2")
            V2 = cpool.tile([P, 2 * HALF], BF16, name="V2")
            CC = cpool.tile([P, 256], F32, name="CC")
            accH = [cpool.tile([P, BINS], F32, name=f"accH{h}") for h in range(2)]
            ghr = cpool.tile([1, 256], F32, name="ghr")
            gh = cpool.tile([1, 256], F32, name="gh")

            for rep in range(1):
                # ---------------- Phase A: global min/max ----------------
                with tc.tile_pool(name=f"pa{rep}", bufs=1) as pa:
                    CW = min(8192, ftot)
                    rmin = pa.tile([P, 1], F32, name="rmin")
                    rmax = pa.tile([P, 1], F32, name="rmax")
                    nc.vector.memset(rmin[:], 1.0e30)
                    nc.vector.memset(rmax[:], -1.0e30)
                    chs = pa.tile([P, CW], F32, name="chs")
                    che = pa.tile([P, CW], F32, name="che")
                    tmin = pa.tile([P, 1], F32, name="tmin")
                    tmax = pa.tile([P, 1], F32, name="tmax")
                    with tc.For_i(0, ftot, CW, name=f"mm{rep}") as cv:
                        nc.sync.dma_start(chs[:], sim_ext[:, bass.ds(cv, CW)])
                        nc.sync.dma_start(che[:], exp_ext[:, bass.ds(cv, CW)])
                        for krep in range(R_PA):
                            for ch in (chs, che):
                                nc.vector.tensor_reduce(
                                    tmin[:], ch[:], mybir.AxisListType.X, AL.min)
                                nc.vector.tensor_reduce(
                                    tmax[:], ch[:], mybir.AxisListType.X, AL.max)
                                nc.vector.tensor_tensor(
                                    rmin[:], rmin[:], tmin[:], AL.min)
                                nc.vector.tensor_tensor(
                                    rmax[:], rmax[:], tmax[:], AL.max)
                    pm = pa.tile([1, 2 * P], F32, name="pm")
                    nc.gpsimd.dma_start(pm[0:1, 0:P], rmax[:, 0:1])
                    nc.gpsimd.dma_start(pm[0:1, P:2 * P], rmin[:, 0:1])
                    pk = pa.tile([1, 2], F32, name="pk")
                    nc.vector.tensor_reduce(
                        pk[0:1, 0:1], pm[0:1, 0:P], mybir.AxisListType.X, AL.max)
                    nc.vector.tensor_reduce(
                        pk[0:1, 1:2], pm[0:1, P:2 * P], mybir.AxisListType.X, AL.min)
                    nc.vector.tensor_scalar_mul(pk[0:1, 1:2], pk[0:1, 1:2], -1.0)
                    nc.gpsimd.dma_start(cc_a_in[:], pk[:])
                    nc.gpsimd.collective_compute(
                        "AllReduce", AL.max, replica_groups=[core_ids],
                        ins=[cc_a_in.opt()], outs=[cc_a_out.opt()],
                    )
                    ga = pa.tile([1, 2], F32, name="ga", bufs=1)
                    nc.gpsimd.dma_start(ga[:], cc_a_out[:])
                    # ga = [mx, -mn]
                    nc.vector.tensor_scalar_mul(sc[0:1, 0:1], ga[0:1, 1:2], -1.0)
                    d_t = pa.tile([1, 1], F32, name="d_t", bufs=1)
                    nc.vector.tensor_tensor(d_t[:], ga[0:1, 0:1], sc[0:1, 0:1], AL.subtract)
                    nc.vector.tensor_scalar_mul(
                        sc[0:1, 1:2], d_t[:], float(np.float32(1.0) / np.float32(127.0)))
                    nc.vector.reciprocal(sc[0:1, 2:3], sc[0:1, 1:2])
                    nc.vector.scalar_tensor_tensor(
                        sc[0:1, 3:4], sc[0:1, 0:1], -1.0, sc[0:1, 2:3],
                        AL.mult, AL.mult)
                    nc.vector.tensor_tensor(
                        sc[0:1, 4:5], sc[0:1, 0:1], sc[0:1, 1:2], AL.add)
                    nc.vector.tensor_scalar_mul(sc[0:1, 5:6], d_t[:], 0.0078125)
                    nc.tensor.matmul(bcps[:], ones1[:], sc[0:1, :],
                                     start=True, stop=True)
                    nc.vector.tensor_copy(bc[:], bcps[:])

                # ---------------- Phase B: mask-reduce histograms ----------------
                for ai, (arr, weighted) in enumerate(
                        ((sim_ext, True), (exp_ext, False))):
                    with tc.tile_pool(name=f"pp{rep}_{ai}", bufs=1) as pp:
                        t = lambda nm: pp.tile([P, FC], F32, name=nm, tag=nm)
                        x = t("x")
                        wt = t("wt")
                        u = t("u")
                        kc = t("kc")
                        s1 = t("s1")
                        s2 = t("s2")
                        hk = t("hk")
                        mp = t("mp")
                        mm = t("mm")
                        cp = t("cp")
                        cm = t("cm")
                        bst = lambda nm: pp.tile([P, FC], BF16, name=nm, tag=nm)
                        vps = bst("vps")
                        vms = bst("vms")
                        khs = bst("khs")
                        kls = bst("kls")
                        with tc.For_i(0, HALF, FC, name=f"prep{rep}_{ai}") as iv:
                            nc.sync.dma_start(x[:], arr[:, bass.ds(iv, FC)])
                            if weighted:
                                nc.sync.dma_start(wt[:], w_ext[:, bass.ds(iv, FC)])
                            # body replicated (idempotent) for R-slope timing
                            for krep in range(R_PREP):
                                nc.scalar.activation(
                                    u[:], x[:], ACT.Identity,
                                    bias=bc[:, 3:4], scale=bc[:, 2:3])
                                nc.vector.tensor_scalar(
                                    kc[:], u[:], MAGIC, -MAGIC, AL.add, AL.add)
                                nc.vector.tensor_tensor(s1[:], kc[:], u[:], AL.is_gt)
                                nc.vector.tensor_tensor(kc[:], kc[:], s1[:], AL.subtract)
                                nc.vector.tensor_scalar(
                                    kc[:], kc[:], 0.0, 126.0, AL.max, AL.min)
                                nc.scalar.activation(
                                    hk[:], kc[:], ACT.Identity,
                                    bias=bc[:, 0:1], scale=bc[:, 1:2])
                                nc.vector.tensor_tensor(s1[:], x[:], hk[:], AL.is_ge)
                                nc.vector.scalar_tensor_tensor(
                                    s2[:], hk[:], bc[:, 1:2], x[:], AL.add, AL.is_gt)
                                nc.vector.tensor_tensor(s1[:], s1[:], s2[:], AL.mult)
                                nc.vector.scalar_tensor_tensor(
                                    mp[:], kc[:], 125.5, s1[:], AL.is_lt, AL.mult)
                                nc.vector.scalar_tensor_tensor(
                                    mm[:], kc[:], 0.5, s1[:], AL.is_gt, AL.mult)
                                nc.vector.tensor_tensor(cp[:], x[:], hk[:], AL.subtract)
                                if weighted:
                                    nc.vector.tensor_tensor(cp[:], cp[:], wt[:], AL.mult)
                                nc.vector.tensor_tensor(vps[:], cp[:], mp[:], AL.mult)
                                nc.vector.scalar_tensor_tensor(
                                    cm[:], hk[:], bc[:, 1:2], x[:], AL.add, AL.subtract)
                                if weighted:
                                    nc.vector.tensor_tensor(cm[:], cm[:], wt[:], AL.mult)
                                nc.vector.tensor_tensor(vms[:], cm[:], mm[:], AL.mult)
                                nc.vector.tensor_scalar_add(khs[:], kc[:], 1.0)
                                nc.vector.tensor_copy(kls[:], kc[:])
                            nc.vector.tensor_copy(V2[:, bass.ds(iv, FC)], vps[:])
                            nc.vector.tensor_copy(V2[:, bass.ds(iv + HALF, FC)], vms[:])
                            nc.vector.tensor_copy(K2[:, bass.ds(iv, FC)], khs[:])
                            nc.vector.tensor_copy(K2[:, bass.ds(iv + HALF, FC)], kls[:])
                    with tc.tile_pool(name=f"pj{rep}_{ai}", bufs=1) as pj:
                        junk = pj.tile([P, HALF], BF16, name="junk")
                        bcs = pj.tile([P, 1], F32, name="bcs")
                        as0 = pj.tile([P, 1], F32, name="as0")
                        as1 = pj.tile([P, 1], F32, name="as1")
                        nc.vector.memset(accH[0][:], 0.0)
                        nc.vector.memset(accH[1][:], 0.0)
                        with tc.For_i(0, BINS, name=f"bins{rep}_{ai}") as bv:
                            nc.vector.tensor_copy(bcs[:], bcol[:, bass.ds(bv, 1)])
                            for krep in range(R_BINS):
                                nc.vector.scalar_tensor_tensor(
                                    junk[:], K2[:, 0:HALF], bcs[:, 0:1],
                                    V2[:, 0:HALF], AL.is_equal, AL.mult,
                                    accum_out=as0[:, 0:1])
                                nc.vector.scalar_tensor_tensor(
                                    junk[:], K2[:, HALF:2 * HALF], bcs[:, 0:1],
                                    V2[:, HALF:2 * HALF], AL.is_equal, AL.mult,
                                    accum_out=as1[:, 0:1])
                            nc.vector.tensor_copy(accH[0][:, bass.ds(bv, 1)], as0[:])
                            nc.vector.tensor_copy(accH[1][:, bass.ds(bv, 1)], as1[:])
                    nc.vector.tensor_tensor(
                        CC[:, ai * BINS:(ai + 1) * BINS],
                        accH[0][:], accH[1][:], AL.add)
                # partition collapse: [1,256] = ones.T @ CC
                nc.tensor.matmul(ps1[:], ones128[:], CC[:],
                                 start=True, stop=True)
                nc.vector.tensor_copy(ghr[:], ps1[:])

                # ---------------- Phase C: all-reduce + chi2 ----------------
                with tc.tile_pool(name=f"pc{rep}", bufs=1) as pc:
                    nc.gpsimd.dma_start(cc_h_in[:], ghr[:])
                    nc.gpsimd.collective_compute(
                        "AllReduce", AL.add, replica_groups=[core_ids],
                        ins=[cc_h_in.opt()], outs=[cc_h_out.opt()],
                    )
                    nc.gpsimd.dma_start(gh[:], cc_h_out[:])
                    for ai in range(2):
                        hist = gh[0:1, ai * BINS:(ai + 1) * BINS]
                        ssum = pc.tile([1, 1], F32, name=f"ssum{ai}")
                        nc.vector.tensor_reduce(
                            ssum[:], hist, mybir.AxisListType.X, AL.add)
                        nc.vector.tensor_tensor(ssum[:], ssum[:], sc[0:1, 5:6], AL.mult)
                        nc.vector.reciprocal(ssum[:], ssum[:])
                        nc.vector.tensor_scalar(
                            hist, hist, ssum[0:1, 0:1], None, AL.mult)
                    dif = pc.tile([1, BINS], F32, name="dif")
                    nc.vector.tensor_tensor(
                        dif[:], gh[0:1, 0:BINS], gh[0:1, BINS:2 * BINS], AL.subtract)
                    nc.vector.tensor_tensor(dif[:], dif[:], dif[:], AL.mult)
                    chi = pc.tile([1, 1], F32, name="chi")
                    nc.vector.tensor_reduce(
                        chi[:], dif[:], mybir.AxisListType.X, AL.add)
                    nc.gpsimd.dma_start(out_ext[:], chi[:])

    _split_sync_waits(nc, __import__("concourse.mybir", fromlist=["x"]),
                      strip_same_engine=strip_waits)
    return nc


_CACHE = {}


def _get_nc(repeat):
    import json
    rp = os.environ.get("BASS_HIST_RP")
    rb = os.environ.get("BASS_HIST_RB")
    ra = os.environ.get("BASS_HIST_RA")
    key = (repeat, rp, rb, ra)
    if key not in _CACHE:
        _CACHE[key] = build(
            repeat=repeat,
            repeat_prep=int(rp) if rp else None,
            repeat_bins=int(rb) if rb else None,
            repeat_pa=int(ra) if ra else None)
    return _CACHE[key]


def kernel(**inputs):
    sim = np.ascontiguousarray(inputs["sim_observable"], dtype=np.float32)
    exp = np.ascontiguousarray(inputs["exp_observable"], dtype=np.float32)
    w = np.ascontiguousarray(inputs["weights"], dtype=np.float32)
    assert sim.shape == (N,) and exp.shape == (N,) and w.shape == (N,)

    from concourse.bass_utils import run_bass_kernel_spmd

    repeat = int(os.environ.get("BASS_HIST_REPEAT", "1"))
    nc = _get_nc(repeat)
    sim_s = sim.reshape(NCORES, P, FTOT_FULL)
    exp_s = exp.reshape(NCORES, P, FTOT_FULL)
    w_s = w.reshape(NCORES, P, FTOT_FULL)
    in_maps = [
        {"sim": sim_s[c], "exp": exp_s[c], "w": w_s[c]} for c in range(NCORES)
    ]
    res = run_bass_kernel_spmd(nc, in_maps, list(range(NCORES)))
    val = res.results[0]["out"][0, 0]
    return np.asarray(val, dtype=np.float32).reshape(())
